# revision 25
# baseline (speedup 1.0000x reference)
"""MoE grouped-GEMM expert FFN (SwiGLU) on 8 Trainium2 NeuronCores.

Expert-parallel sharding: tokens arrive pre-grouped by expert with uniform
group size g = T/E = 1024, so core c owns experts [4c, 4c+4) and token rows
[c*4096, (c+1)*4096). No cross-core communication is needed: each core
computes its own 4 experts' FFN on its own token block.

Per-core math, per expert e:
    gu^T = w13_e^T-chunks @ x_e^T        # PE: contract H on partitions
    h^T  = silu(gate^T) * up^T           # ACT (Silu) + DVE (mul), bf16
    out  = h @ w2_e                      # PE: contract I on partitions

Dataflow: w13 is host-packed into use-once streaming tiles (one per
(expert, m-tile): [128, 16k, 256]) so each 1MB pack is DMA'd just-in-time
and released right after its k-loop; w2 and x^T are whole-expert resident,
double-buffered across experts. All loads are split into ~512KB dma_starts
(one DMA engine each, ~22.5GB/s) so aggregate bandwidth comes from
concurrency. Phase-1 interleaves gate/up inside the k-loop (banks
pg0/pg1/pu0/pu1, one stationary per 2 matmuls); phase-2 feeds 4 matmuls per
stationary h-tile (banks po0..3). Output is written bf16 (halves the write
traffic) and upcast to f32 on the host.

Post-compile passes: dedup_ldweights removes the redundant per-matmul weight
reloads bass emits when consecutive matmuls share a stationary;
coalesce_pe_incs batches per-matmul S[PE]++ updates into one +=n per
wait-free run. Measured per-MM cost is ~257ns (512 cols at the ~2.0GHz
sustained-load PE clock), which puts the 3072-matmul kernel at its compute
floor (~790us/core marginal); DMA (~230us) and epilogues fully overlap.
"""

import sys

if "/opt/trn_rl_repo" not in sys.path:
    sys.path.insert(0, "/opt/trn_rl_repo")

import ml_dtypes
import numpy as np

import concourse.bacc as bacc
import concourse.mybir as mybir
from concourse import tile
from concourse.bass_utils import run_bass_kernel_spmd

BF16 = mybir.dt.bfloat16
F32 = mybir.dt.float32
NPBF16 = ml_dtypes.bfloat16

N_CORES = 8
E = 32
H = 2048
I = 1024
T = 32768
EPC = E // N_CORES          # experts per core = 4
G = T // E                  # tokens per expert = 1024
ROWS = EPC * G              # token rows per core = 4096
KH = H // 128               # 16 contraction tiles for GEMM1
KI = I // 128               # 8 contraction tiles for GEMM2
NH = H // 512               # 4 H-chunks for GEMM2 output


def _ldw_sig(inst):
    return (str(inst.ins[0]), str(inst.tile_position), str(inst.tile_size),
            str(inst.perf_mode), str(inst.is_transpose))


def dedup_ldweights(nc):
    """Remove InstLdweights that reload the exact weights already resident in
    the PE array (bass emits one per matmul; consecutive matmuls sharing a
    stationary tile don't need the reload). A redundant load's waits are moved
    onto the following Matmult when that slot is free (engine instructions
    carry at most one wait); otherwise the load is kept as a wait carrier."""
    removed = 0
    for blk in nc.m.functions[0].blocks:
        insts = blk.instructions
        keep = []
        last_sig = None
        pending_del = None
        for inst in insts:
            tn = type(inst).__name__
            if tn == "InstLdweights":
                sig = _ldw_sig(inst)
                si = inst.sync_info
                waits = list(si.on_wait) if si is not None else []
                upds = list(si.on_update) if si is not None else []
                if sig == last_sig and not upds:
                    if not waits:
                        removed += 1
                        continue
                    # try to push the waits onto the next PE instruction
                    pending_del = (inst, waits)
                    continue
                last_sig = sig
                keep.append(inst)
            elif tn == "InstMatmult":
                if pending_del is not None:
                    dinst, waits = pending_del
                    msi = inst.sync_info
                    mwaits = list(msi.on_wait) if msi is not None else []
                    if not mwaits:
                        inst.sync_info = mybir.SyncInfo(
                            on_wait=waits,
                            on_update=list(msi.on_update) if msi is not None else [],
                        )
                        removed += 1
                    else:
                        keep.append(dinst)
                    pending_del = None
                keep.append(inst)
            else:
                if pending_del is not None:
                    keep.append(pending_del[0])
                    pending_del = None
                if inst.engine == mybir.EngineType.PE and tn not in (
                        "InstEventSemaphore", "InstNop"):
                    # unknown PE instruction (branch/drain): invalidate
                    last_sig = None
                keep.append(inst)
        if pending_del is not None:
            keep.append(pending_del[0])
        if len(keep) != len(insts):
            blk.instructions = keep
    return removed


def coalesce_pe_incs(nc):
    """Coalesce per-matmul `S[PE]++1` updates into one `+=n` on the last
    instruction of each wait-free PE run. Sem increments serialize on the
    EVT_SEM register (~26ns each); 3072 of them cost real PE time. Deferring
    an increment to the end of a wait-free run is deadlock-safe: the PE
    instructions in between wait on nothing, so no cycle can form, and every
    deferred increment lands before the next PE wait."""
    coalesced = 0
    for blk in nc.m.functions[0].blocks:
        run = []          # (inst, inc_value) entries with strippable updates
        run_sem = None

        def flush():
            nonlocal run, run_sem, coalesced
            if len(run) > 1:
                total = sum(v for _, v in run)
                for inst, _ in run[:-1]:
                    si = inst.sync_info
                    inst.sync_info = mybir.SyncInfo(
                        on_wait=list(si.on_wait), on_update=[])
                last = run[-1][0]
                si = last.sync_info
                upd = mybir.SyncUpdate(
                    sync_type="semaphore", id=run_sem[0],
                    update_mode="sem-inc" if total == 1 else "sem-add-imm",
                    update_value=total, ant_name=run_sem[1])
                last.sync_info = mybir.SyncInfo(
                    on_wait=list(si.on_wait) if si else [], on_update=[upd])
                coalesced += len(run) - 1
            run = []
            run_sem = None

        for inst in blk.instructions:
            if inst.engine != mybir.EngineType.PE:
                continue
            tn = type(inst).__name__
            si = inst.sync_info
            waits = list(si.on_wait) if si is not None else []
            upds = list(si.on_update) if si is not None else []
            if tn not in ("InstMatmult", "InstLdweights"):
                flush()
                continue
            if waits:
                flush()
            if len(upds) == 1 and upds[0].update_mode == "sem-inc":
                u = upds[0]
                key = (u.id, u.ant_name)
                if run_sem is not None and key != run_sem:
                    flush()
                run_sem = key
                run.append((inst, u.update_value))
                # accumulation-group ends gate PSUM evacuation on other
                # engines; release their counts immediately
                if tn == "InstMatmult" and inst.stop_tensor_calc:
                    flush()
            elif upds:
                flush()
        flush()
    return coalesced


def build_nc(reps=1, dedup=True, coalesce=True):
    nc = bacc.Bacc()
    xt_d = nc.declare_dram_parameter("xt", [EPC, 128, KH, G], BF16, isOutput=False)
    w13_d = nc.declare_dram_parameter("w13", [EPC, KI, 128, KH, 256], BF16, isOutput=False)
    w2_d = nc.declare_dram_parameter("w2", [EPC, 128, KI, H], BF16, isOutput=False)
    out_d = nc.declare_dram_parameter("out", [ROWS, H], BF16, isOutput=True)

    with tile.TileContext(nc) as tc:
        with (
            tc.tile_pool(name="xt", bufs=2) as xt_pool,
            tc.tile_pool(name="w13", bufs=4) as w13_pool,
            tc.tile_pool(name="w2", bufs=2) as w2_pool,
            tc.tile_pool(name="h", bufs=2) as h_pool,
            tc.tile_pool(name="tmp", bufs=3) as tmp_pool,
            tc.tile_pool(name="ost", bufs=6) as ost_pool,
            tc.tile_pool(name="ps", bufs=1, space="PSUM") as ps_pool,
        ):
            for it in range(EPC * reps):
                e = it % EPC
                # Split big loads into ~512KB pieces: each dma_start rides one
                # DMA engine (~22.5GB/s), so aggregate bandwidth comes from
                # many concurrent transfers.
                xt = xt_pool.tile([128, KH, G], BF16, tag="xt", bufs=2, name=f"xt_{it}")
                # Entry-order the loads by first use: the first m-group's
                # k-loop starts after xt k0-1 and the m=0 w13 pack, so those
                # pieces issue ahead of the remaining 3MB of xt — as single
                # k-tiles, so the first arrives in half the time.
                nc.sync.dma_start(xt[:, 0:1, :], xt_d[e][:, 0:1, :])
                nc.sync.dma_start(xt[:, 1:2, :], xt_d[e][:, 1:2, :])
                w13m0 = w13_pool.tile([128, KH, 256], BF16, tag="w13", bufs=4,
                                      name=f"w13_{it}_0")
                for k4 in range(0, KH, 4):
                    nc.sync.dma_start(w13m0[:, k4:k4 + 4, :],
                                      w13_d[e, 0][:, k4:k4 + 4, :])
                for k2 in range(2, KH, 2):
                    nc.sync.dma_start(xt[:, k2:k2 + 2, :], xt_d[e][:, k2:k2 + 2, :])
                w2e = w2_pool.tile([128, KI, H], BF16, tag="w2", bufs=2, name=f"w2_{it}")

                # Phase 1: gu^T tiles -> SwiGLU -> h^T resident in SBUF (bf16).
                h_sb = [h_pool.tile([128, G], BF16, tag=f"h{m}", bufs=2, name=f"h{m}_{it}")
                        for m in range(KI)]
                for m in range(KI):
                    if m == 0:
                        w13m = w13m0
                    else:
                        w13m = w13_pool.tile([128, KH, 256], BF16, tag="w13", bufs=4,
                                             name=f"w13_{it}_{m}")
                        for k4 in range(0, KH, 4):
                            nc.sync.dma_start(w13m[:, k4:k4 + 4, :],
                                              w13_d[e, m][:, k4:k4 + 4, :])
                    if m == 1:
                        # w2 loads issue after the first two w13 packs so
                        # kernel-entry DMA bandwidth goes to phase 1 first;
                        # still a whole phase-1 of prefetch cover for phase 2.
                        for k in range(KI):
                            nc.sync.dma_start(w2e[:, k, :], w2_d[e][:, k, :])
                    pg = [ps_pool.tile([128, 512], F32, tag=f"pg{n}", bufs=1,
                                       name=f"pg{n}_{it}_{m}") for n in range(2)]
                    pu = [ps_pool.tile([128, 512], F32, tag=f"pu{n}", bufs=1,
                                       name=f"pu{n}_{it}_{m}") for n in range(2)]
                    # Interleave gate/up inside the k loop: each stationary
                    # feeds two consecutive matmuls (LDW reuse) and the PE
                    # pipeline fills across the four banks.
                    for k in range(KH):
                        wg = w13m[:, k, 0:128]
                        wu = w13m[:, k, 128:256]
                        for n in range(2):
                            nc.tensor.matmul(
                                pg[n][:], wg, xt[:, k, n * 512:(n + 1) * 512],
                                start=(k == 0), stop=(k == KH - 1),
                            )
                        for n in range(2):
                            nc.tensor.matmul(
                                pu[n][:], wu, xt[:, k, n * 512:(n + 1) * 512],
                                start=(k == 0), stop=(k == KH - 1),
                            )
                    for n in range(2):
                        ncol = slice(n * 512, (n + 1) * 512)
                        tmp = tmp_pool.tile([128, 512], BF16, tag="tmp", bufs=3,
                                            name=f"tmp_{it}_{m}_{n}")
                        pu_sb = tmp_pool.tile([128, 512], BF16, tag="pusb", bufs=3,
                                              name=f"pusb_{it}_{m}_{n}")
                        nc.scalar.activation(
                            tmp[:], pg[n][:], mybir.ActivationFunctionType.Silu
                        )
                        # Both epilogue producers run on ACT so the DVE mul
                        # carries ONE merged ACT wait (the TT instruction
                        # encoding only fits a single sync-wait).
                        nc.scalar.copy(pu_sb[:], pu[n][:])
                        nc.vector.tensor_mul(h_sb[m][:, ncol], tmp[:], pu_sb[:])

                # Phase 2: out_e = h @ w2_e, streamed straight to DRAM.
                # One stationary h-tile feeds 4 matmuls (the 4 H-chunks).
                for mt in range(KI):
                    rows = slice(e * G + mt * 128, e * G + (mt + 1) * 128)
                    po = [ps_pool.tile([128, 512], F32, tag=f"po{n}", bufs=1,
                                       name=f"po{n}_{it}_{mt}") for n in range(4)]
                    for k in range(KI):
                        hk = h_sb[k][:, mt * 128:(mt + 1) * 128]
                        for n in range(4):
                            nc.tensor.matmul(
                                po[n][:], hk, w2e[:, k, n * 512:(n + 1) * 512],
                                start=(k == 0), stop=(k == KI - 1),
                            )
                    for n in range(4):
                        ncol = slice(n * 512, (n + 1) * 512)
                        ot = ost_pool.tile([128, 512], BF16, tag="ot", bufs=6,
                                           name=f"ot_{it}_{mt}_{n}")
                        nc.vector.tensor_copy(ot[:], po[n][:])
                        nc.sync.dma_start(out_d[rows, ncol], ot[:])
    nc.compile()
    if dedup:
        dedup_ldweights(nc)
    if coalesce:
        coalesce_pe_incs(nc)
    return nc


def _in_map_for_core(x, w13, w2, c):
    xs = x[c * ROWS:(c + 1) * ROWS]                      # [4096, 2048] f32
    # xt[e, p, k, g] = x[e*G + g, k*128 + p]
    xt = (xs.reshape(EPC, G, KH, 128).transpose(0, 3, 2, 1)
          .astype(NPBF16, order="C"))
    w13c = w13[c * EPC:(c + 1) * EPC]                    # [EPC, 2048, 2048]
    w13r = w13c.reshape(EPC, KH, 128, 2 * I)
    gate = w13r[:, :, :, :I].reshape(EPC, KH, 128, KI, 128)
    up = w13r[:, :, :, I:].reshape(EPC, KH, 128, KI, 128)
    # w13p[e, m, p, k, 0:128]=gate[e,k,p,m,:], [...,128:256]=up[e,k,p,m,:]
    w13p = np.concatenate([gate, up], axis=-1)           # [EPC, KH, 128, KI, 256]
    w13p = w13p.transpose(0, 3, 2, 1, 4).astype(NPBF16, order="C")
    w2c = w2[c * EPC:(c + 1) * EPC]                      # [EPC, 1024, 2048]
    w2r = w2c.reshape(EPC, KI, 128, H)
    # w2p[e, p, k, :] = w2[e, k*128+p, :]
    w2p = w2r.transpose(0, 2, 1, 3).astype(NPBF16, order="C")
    return {"xt": xt, "w13": w13p, "w2": w2p}


_NC_CACHE = None


def kernel(x, w13, w2, tokens_per_expert, decoding, _trace=False):
    global _NC_CACHE
    x = np.asarray(x, dtype=np.float32)
    w13 = np.asarray(w13, dtype=np.float32)
    w2 = np.asarray(w2, dtype=np.float32)

    in_maps = [_in_map_for_core(x, w13, w2, c) for c in range(N_CORES)]
    if _NC_CACHE is None:
        _NC_CACHE = build_nc()
    nc = _NC_CACHE
    res = run_bass_kernel_spmd(nc, in_maps, list(range(N_CORES)), trace=_trace)
    out = np.concatenate(
        [res.results[c]["out"].astype(np.float32) for c in range(N_CORES)], axis=0
    )
    if _trace:
        return out, res
    return out
